# revision 15
# baseline (speedup 1.0000x reference)
"""Expert-parallel MoE MLP Bass kernel for TRN2 with Strassen-1 on GEMM1.

Per-core: y[tok,1024] = W2 @ gelu(W1 @ x + b1) + b2, 2048 tok, d_hid 4096.

GEMM1 (fc1) uses one Strassen level over (M=4096, K=1024, N=2048tok):
7/8 of the fp16 matmul work. Host precomputes the 7 stationary operands
S_p; DVE computes the 5 moving-side combos from x; the 7 products live in
7 PSUM banks; DVE combines them into the 4 output quadrants; ScalarE
applies gelu+bias into the shared h buffer. GEMM2 (fc2) is a plain
k-accumulated GEMM with streamed weights (eighth PSUM bank).

Token dim processed in two 1024-halves j=0,1 (Strassen N-split pairs
column block j of each half). PE floor: (896 + 1024) * 216.3 = 415us.
"""
import sys

sys.path.insert(0, "/opt/trn_rl_repo")

import numpy as np

import concourse.bass as bass  # noqa: F401
import concourse.tile as tile
from concourse import bacc, mybir
from concourse.bass_utils import run_bass_kernel_spmd

E = 8
T_PER_E = 2048
D_IN = 1024
D_HID = 4096
D_OUT = 1024

CDT = mybir.dt.float16
NP_CDT = np.float16
F32 = mybir.dt.float32

N_WARM = 34   # sized to keep the PE (and HAM activity window) busy until
              # the first real tiles land (~14.5us); undershooting strands
              # the clock at a low p-state for the whole run

NI1 = 16            # m-tiles of M/2=2048 (GEMM1 Strassen)
NK1 = 4             # k-tiles of K/2=512
ND2 = 8             # d-tiles of D_OUT (GEMM2 plain)
NK2 = 32            # k-tiles of D_HID

W1S_ICOLS = 7 * NK1 * 128       # cols per single i (=3584)
W1S_CHUNK = 4 * W1S_ICOLS       # cols per 4-i chunk (=14336)

_compiled = None


def _build():
    nc = bacc.Bacc("TRN2", target_bir_lowering=False, debug=False)

    # Host layouts:
    #  w1S[p, i(16), P(7), k(4), m(128)] = S_P[i*128+m, k*128+p]
    #  w2L[p, d(8), k(32), m(128)]      = w2[d*128+m, k*128+p]
    #  xL [p, j(2), piece(4), kk(4), c(512)]: piece 0=B11,1=B22,2=B12,3=B21
    #  bb [p, 0:32]=b1 tiles, [p, 32:40]=b2 tiles
    w1S = nc.dram_tensor("w1S", [128, 16 * W1S_ICOLS], CDT, kind="ExternalInput").ap()
    w2L = nc.dram_tensor("w2L", [128, ND2 * NK2 * 128], CDT, kind="ExternalInput").ap()
    xL = nc.dram_tensor("xL", [128, 2 * 4 * 4 * 512], CDT, kind="ExternalInput").ap()
    bb = nc.dram_tensor("bb", [128, 40], F32, kind="ExternalInput").ap()
    yT = nc.dram_tensor("yT", [D_OUT, T_PER_E], CDT, kind="ExternalOutput").ap()

    GELU = mybir.ActivationFunctionType.Gelu
    IDENT = mybir.ActivationFunctionType.Identity

    with tile.TileContext(nc) as tc:
        with tc.tile_pool(name="wc", bufs=4) as wc_pool, \
             tc.tile_pool(name="w2c", bufs=2) as w2c_pool, \
             tc.tile_pool(name="xp", bufs=2) as xp, \
             tc.tile_pool(name="xcp", bufs=1) as xcp, \
             tc.tile_pool(name="hp", bufs=1) as hp, \
             tc.tile_pool(name="sg", bufs=2) as sg, \
             tc.tile_pool(name="sgt", bufs=1) as sgt, \
             tc.tile_pool(name="op", bufs=4) as op_pool, \
             tc.tile_pool(name="scr", bufs=1) as scrp, \
             tc.tile_pool(name="ps", bufs=1, space="PSUM") as ps:

            bb_sb = scrp.tile([128, 40], F32, name="bb_sb")
            scr = scrp.tile([128, 256], CDT, name="scr")
            nc.vector.memset(scr[:], 0.0)
            for i in range(N_WARM):
                wps = ps.tile([128, 256], F32, tag=f"m{i % 7}", name=f"warm{i}")
                nc.tensor.matmul(wps[:], scr[:, :128], scr[:], start=True, stop=True)

            # h buffer: [p, r(32), q(2), c(512)] fp16 (one j-phase of tokens)
            hq = hp.tile([128, 32, 2, 512], CDT, name="hq")

            def load_x(j, first):
                xr = xp.tile([128, 4, 4, 512], CDT, tag="xr", name=f"xr{j}")
                xb = j * 8192
                nc.scalar.dma_start(xr[:, 0], xL[:, xb:xb + 2048])
                nc.scalar.dma_start(xr[:, 1], xL[:, xb + 2048:xb + 4096])
                nc.scalar.dma_start(xr[:, 2], xL[:, xb + 4096:xb + 6144])
                nc.scalar.dma_start(xr[:, 3], xL[:, xb + 6144:xb + 8192])
                if first:
                    # bb is only needed by the first ACT (~20us in) — queue it
                    # behind the x pieces that gate the first matmuls
                    nc.scalar.dma_start(bb_sb[:], bb[:, :])
                # x combos [p, t(5), kk(4), c] : T1,T3,T4,T6,T7 (T1 first)
                xc = xcp.tile([128, 5, 4, 512], CDT, tag="xc", name=f"xc{j}")
                for k in range(4):
                    nc.vector.tensor_add(xc[:, 0, k], xr[:, 0, k], xr[:, 1, k])
                for k in range(4):
                    nc.vector.tensor_sub(xc[:, 1, k], xr[:, 2, k], xr[:, 1, k])
                    nc.vector.tensor_sub(xc[:, 2, k], xr[:, 3, k], xr[:, 0, k])
                    nc.vector.tensor_add(xc[:, 3, k], xr[:, 0, k], xr[:, 2, k])
                    nc.vector.tensor_add(xc[:, 4, k], xr[:, 3, k], xr[:, 1, k])
                return xr, xc

            def gemm1(j, xr, xc):
                def mov1(P, k):
                    return [xc[:, 0, k], xr[:, 0, k], xc[:, 1, k], xc[:, 2, k],
                            xr[:, 1, k], xc[:, 3, k], xc[:, 4, k]][P]

                # per-i weight tiles (7KB): fine-grained stream, first two
                # i's striped onto the scalar ring for a fast head
                w_tiles = []
                for i in range(NI1):
                    wt = wc_pool.tile([128, 7, 4, 128], CDT, tag="w1c",
                                      name=f"w1c_{j}_{i}")
                    base = i * W1S_ICOLS
                    eng = nc.scalar if (j == 0 and i in (1, 3)) else nc.sync
                    eng.dma_start(wt[:], w1S[:, base:base + W1S_ICOLS])
                    w_tiles.append(wt)

                for i in range(NI1):
                    wt = w_tiles[i]
                    # on the very first tile, run the products that only need
                    # the early x pieces (T1/B11/B22) before the ones gated on
                    # the later xC/xD transfers
                    p_order = ((0, 1, 4, 2, 3, 5, 6) if (j == 0 and i == 0)
                               else range(7))
                    ms = [None] * 7
                    for P in p_order:
                        psum = ps.tile([128, 512], F32, tag=f"m{P}",
                                       name=f"g1_{j}_{i}_{P}")
                        for k in range(NK1):
                            nc.tensor.matmul(psum[:], wt[:, P, k], mov1(P, k),
                                             start=(k == 0), stop=(k == NK1 - 1))
                        ms[P] = psum
                    stA = sg.tile([128, 512], F32, tag="stA", name=f"stA{j}_{i}")
                    stB = sg.tile([128, 512], F32, tag="stB", name=f"stB{j}_{i}")
                    stC = sg.tile([128, 512], F32, tag="stC", name=f"stC{j}_{i}")
                    stD = sg.tile([128, 512], F32, tag="stD", name=f"stD{j}_{i}")
                    t0 = sgt.tile([128, 512], F32, tag="t0", name=f"t0_{j}_{i}")
                    t3 = sgt.tile([128, 512], F32, tag="t3", name=f"t3_{j}_{i}")
                    t4 = sgt.tile([128, 512], F32, tag="t4", name=f"t4_{j}_{i}")
                    # Any engine reads at most one PSUM input per op; copies
                    # of the twice-used products first, ordered so banks free
                    # in the next i's consumption order (m0 first). t0/t3 go
                    # through the lightly-loaded ScalarE to unload DVE.
                    nc.scalar.activation(t0[:], ms[0][:], IDENT)       # frees m0
                    nc.scalar.activation(t3[:], ms[3][:], IDENT)       # frees m3
                    nc.vector.tensor_copy(t4[:], ms[4][:])             # frees m4
                    nc.vector.tensor_add(stC[:], t3[:], ms[1][:])
                    nc.vector.tensor_sub(stD[:], t0[:], ms[1][:])      # frees m1
                    nc.vector.tensor_add(stB[:], t4[:], ms[2][:])
                    nc.vector.tensor_add(stD[:], stD[:], ms[2][:])     # frees m2
                    nc.vector.tensor_add(stD[:], stD[:], ms[5][:])     # frees m5
                    nc.vector.tensor_add(stA[:], t0[:], t3[:])
                    nc.vector.tensor_sub(stA[:], stA[:], t4[:])
                    nc.vector.tensor_add(stA[:], stA[:], ms[6][:])     # frees m6
                    nc.scalar.activation(hq[:, 16 + i, 0], stC[:], GELU,
                                         bias=bb_sb[:, 16 + i:17 + i], scale=1.0)
                    nc.scalar.activation(hq[:, i, 1], stB[:], GELU,
                                         bias=bb_sb[:, i:i + 1], scale=1.0)
                    nc.scalar.activation(hq[:, 16 + i, 1], stD[:], GELU,
                                         bias=bb_sb[:, 16 + i:17 + i], scale=1.0)
                    nc.scalar.activation(hq[:, i, 0], stA[:], GELU,
                                         bias=bb_sb[:, i:i + 1], scale=1.0)

            def gemm2(j):
                for d in range(ND2):
                    w2t = w2c_pool.tile([128, 32, 128], CDT, tag="w2c",
                                        name=f"w2c_{j}_{d}")
                    nc.scalar.dma_start(
                        w2t[:], w2L[:, d * NK2 * 128:(d + 1) * NK2 * 128])
                    for q in range(2):
                        # alternate with the (idle in this phase) m0 bank so
                        # the next group's start doesn't wait on the ACT read;
                        # the very last group drains in two column chunks
                        last = (j == 1 and d == ND2 - 1 and q == 1)
                        chunks = ((0, 256), (256, 512)) if last else ((0, 512),)
                        for ci, (c0, c1) in enumerate(chunks):
                            even = (d * 2 + q + ci) % 2 == 0
                            psum = ps.tile([128, c1 - c0], F32,
                                           tag=("g2" if even else "m0"),
                                           name=f"g2_{j}_{d}_{q}_{c0}")
                            for k in range(NK2):
                                nc.tensor.matmul(psum[:], w2t[:, k],
                                                 hq[:, k, q, c0:c1],
                                                 start=(k == 0),
                                                 stop=(k == NK2 - 1))
                            o_sb = op_pool.tile([128, c1 - c0], CDT, tag="o",
                                                name=f"o_{j}_{d}_{q}_{c0}")
                            nc.scalar.activation(o_sb[:], psum[:], IDENT,
                                                 bias=bb_sb[:, 32 + d:33 + d],
                                                 scale=1.0)
                            t0 = q * 1024 + j * 512
                            nc.scalar.dma_start(
                                yT[d * 128:(d + 1) * 128,
                                   t0 + c0:t0 + c1], o_sb[:])

            xr0, xc0 = load_x(0, first=True)
            gemm1(0, xr0, xc0)
            xr1, xc1 = load_x(1, first=False)   # hidden under gemm2(0)
            gemm2(0)
            gemm1(1, xr1, xc1)
            gemm2(1)

    nc.compile()
    return nc


def _get_compiled():
    global _compiled
    if _compiled is None:
        _compiled = _build()
    return _compiled


def _make_S(A):
    M, K = A.shape
    A11, A12 = A[:M // 2, :K // 2], A[:M // 2, K // 2:]
    A21, A22 = A[M // 2:, :K // 2], A[M // 2:, K // 2:]
    return np.stack([A11 + A22, A21 + A22, A11, A22,
                     A11 + A12, A21 - A11, A12 - A22]).astype(NP_CDT)


def _make_in_maps(x, w1, b1, w2, b2):
    in_maps = []
    for e in range(E):
        xe = x[e * T_PER_E:(e + 1) * T_PER_E].astype(np.float32)
        S1 = _make_S(w1[e].astype(np.float32))            # [7, 2048, 512]
        w1s = (S1.reshape(7, 16, 128, 4, 128)
               .transpose(4, 1, 0, 3, 2).reshape(128, -1))
        w2l = (w2[e].astype(NP_CDT).reshape(8, 128, 32, 128)
               .transpose(3, 0, 2, 1).reshape(128, -1))
        xq = xe.astype(NP_CDT).reshape(2, 2, 512, 8, 128)  # [q, j, c, k, p]
        pieces = np.stack([xq[0, :, :, 0:4, :], xq[1, :, :, 4:8, :],
                           xq[1, :, :, 0:4, :], xq[0, :, :, 4:8, :]])
        xl = pieces.transpose(4, 1, 0, 3, 2).reshape(128, -1)  # [p,j,pc,kk,c]
        bbe = np.concatenate([b1[e].reshape(32, 128).T,
                              b2[e].reshape(8, 128).T], axis=1).astype(np.float32)
        in_maps.append({
            "w1S": np.ascontiguousarray(w1s),
            "w2L": np.ascontiguousarray(w2l),
            "xL": np.ascontiguousarray(xl),
            "bb": np.ascontiguousarray(bbe),
        })
    return in_maps


def run(x, cnt, w1, b1, w2, b2, trace=False):
    nc = _get_compiled()
    in_maps = _make_in_maps(x, w1, b1, w2, b2)
    res = run_bass_kernel_spmd(nc, in_maps, core_ids=list(range(E)), trace=trace)
    outs = [res.results[e]["yT"].T for e in range(E)]
    y = np.concatenate(outs, axis=0).astype(np.float32)
    return y, res


def kernel(x, cnt, w1, b1, w2, b2):
    y, _ = run(x, cnt, w1, b1, w2, b2, trace=False)
    return y


# revision 22
# speedup vs baseline: 1.0041x; 1.0041x over previous
"""Expert-parallel MoE MLP Bass kernel for TRN2 with Strassen-1 on GEMM1.

Per-core: y[tok,1024] = W2 @ gelu(W1 @ x + b1) + b2, 2048 tok, d_hid 4096.

GEMM1 (fc1) uses one Strassen level over (M=4096, K=1024, N=2048tok):
7/8 of the fp16 matmul work. Host precomputes the 7 stationary operands
S_p; DVE computes the 5 moving-side combos from x; the 7 products live in
7 PSUM banks; DVE combines them into the 4 output quadrants; ScalarE
applies gelu+bias into the shared h buffer. GEMM2 (fc2) is a plain
k-accumulated GEMM with streamed weights (eighth PSUM bank).

Token dim processed in two 1024-halves j=0,1 (Strassen N-split pairs
column block j of each half). PE floor: (896 + 1024) * 216.3 = 415us.
"""
import sys

sys.path.insert(0, "/opt/trn_rl_repo")

import numpy as np

import concourse.bass as bass  # noqa: F401
import concourse.tile as tile
from concourse import bacc, mybir
from concourse.bass_utils import run_bass_kernel_spmd

E = 8
T_PER_E = 2048
D_IN = 1024
D_HID = 4096
D_OUT = 1024

CDT = mybir.dt.float16
NP_CDT = np.float16
F32 = mybir.dt.float32

N_WARM = 34   # sized to keep the PE (and HAM activity window) busy until
              # the first real tiles land (~14.5us); undershooting strands
              # the clock at a low p-state for the whole run

NI1 = 16            # m-tiles of M/2=2048 (GEMM1 Strassen)
NK1 = 4             # k-tiles of K/2=512
ND2 = 8             # d-tiles of D_OUT (GEMM2 plain)
NK2 = 32            # k-tiles of D_HID

W1S_ICOLS = 7 * NK1 * 128       # cols per single i (=3584)
W1S_CHUNK = 4 * W1S_ICOLS       # cols per 4-i chunk (=14336)

_compiled = None


def _build():
    nc = bacc.Bacc("TRN2", target_bir_lowering=False, debug=False)

    # Host layouts:
    #  w1S[p, i(16), P(7), k(4), m(128)] = S_P[i*128+m, k*128+p]
    #  w2L[p, d(8), k(32), m(128)]      = w2[d*128+m, k*128+p]
    #  xL [p, j(2), piece(4), kk(4), c(512)]: piece 0=B11,1=B22,2=B12,3=B21
    #  bb [p, 0:32]=b1 tiles, [p, 32:40]=b2 tiles
    w1S = nc.dram_tensor("w1S", [128, 16 * W1S_ICOLS], CDT, kind="ExternalInput").ap()
    w2L = nc.dram_tensor("w2L", [128, ND2 * NK2 * 128], CDT, kind="ExternalInput").ap()
    xL = nc.dram_tensor("xL", [128, 2 * 4 * 4 * 512], CDT, kind="ExternalInput").ap()
    bb = nc.dram_tensor("bb", [128, 40], F32, kind="ExternalInput").ap()
    wz = nc.dram_tensor("wz", [128, 256], CDT, kind="ExternalInput").ap()
    yT = nc.dram_tensor("yT", [D_OUT, T_PER_E], CDT, kind="ExternalOutput").ap()

    GELU = mybir.ActivationFunctionType.Gelu
    IDENT = mybir.ActivationFunctionType.Identity

    with tile.TileContext(nc) as tc:
        with tc.tile_pool(name="wc", bufs=4) as wc_pool, \
             tc.tile_pool(name="w2c", bufs=2) as w2c_pool, \
             tc.tile_pool(name="xp", bufs=2) as xp, \
             tc.tile_pool(name="xcp", bufs=1) as xcp, \
             tc.tile_pool(name="hp", bufs=1) as hp, \
             tc.tile_pool(name="sg", bufs=2) as sg, \
             tc.tile_pool(name="sgt", bufs=1) as sgt, \
             tc.tile_pool(name="op", bufs=4) as op_pool, \
             tc.tile_pool(name="scr", bufs=1) as scrp, \
             tc.tile_pool(name="ps", bufs=1, space="PSUM") as ps:

            bb_sb = scrp.tile([128, 40], F32, name="bb_sb")
            # warmup operand comes from a DMA'd zero tensor rather than a DVE
            # memset: the DVE queue's boot preamble is several us slower than
            # the DMA rings, and the warmups gate on this write
            scr = scrp.tile([128, 256], CDT, name="scr")
            nc.sync.dma_start(scr[:], wz)
            for i in range(N_WARM):
                wps = ps.tile([128, 256], F32, tag=f"m{i % 7}", name=f"warm{i}")
                nc.tensor.matmul(wps[:], scr[:, :128], scr[:], start=True, stop=True)

            # h buffer: [p, r(32), q(2), c(512)] fp16 (one j-phase of tokens)
            hq = hp.tile([128, 32, 2, 512], CDT, name="hq")

            def load_x(j, first):
                xr = xp.tile([128, 4, 4, 512], CDT, tag="xr", name=f"xr{j}")
                xb = j * 8192
                # pieces 0/1 in k-halves: the first T1 combo ops (and hence
                # the first real matmuls) unblock on half the transfer
                nc.scalar.dma_start(xr[:, 0, 0:2], xL[:, xb:xb + 1024])
                nc.scalar.dma_start(xr[:, 1, 0:2], xL[:, xb + 2048:xb + 3072])
                nc.scalar.dma_start(xr[:, 0, 2:4], xL[:, xb + 1024:xb + 2048])
                nc.scalar.dma_start(xr[:, 1, 2:4], xL[:, xb + 3072:xb + 4096])
                nc.scalar.dma_start(xr[:, 2], xL[:, xb + 4096:xb + 6144])
                nc.scalar.dma_start(xr[:, 3], xL[:, xb + 6144:xb + 8192])
                if first:
                    # bb is only needed by the first ACT (~20us in) — queue it
                    # behind the x pieces that gate the first matmuls
                    nc.scalar.dma_start(bb_sb[:], bb[:, :])
                # x combos [p, t(5), kk(4), c] : T1,T3,T4,T6,T7 (T1 first)
                xc = xcp.tile([128, 5, 4, 512], CDT, tag="xc", name=f"xc{j}")
                for k in range(4):
                    nc.vector.tensor_add(xc[:, 0, k], xr[:, 0, k], xr[:, 1, k])
                for k in range(4):
                    nc.vector.tensor_sub(xc[:, 1, k], xr[:, 2, k], xr[:, 1, k])
                    nc.vector.tensor_sub(xc[:, 2, k], xr[:, 3, k], xr[:, 0, k])
                    nc.vector.tensor_add(xc[:, 3, k], xr[:, 0, k], xr[:, 2, k])
                    nc.vector.tensor_add(xc[:, 4, k], xr[:, 3, k], xr[:, 1, k])
                return xr, xc

            def gemm1(j, xr, xc):
                def mov1(P, k):
                    return [xc[:, 0, k], xr[:, 0, k], xc[:, 1, k], xc[:, 2, k],
                            xr[:, 1, k], xc[:, 3, k], xc[:, 4, k]][P]

                # per-i weight tiles (7KB): fine-grained stream, first two
                # i's striped onto the scalar ring for a fast head
                w_tiles = []
                for i in range(NI1):
                    wt = wc_pool.tile([128, 7, 4, 128], CDT, tag="w1c",
                                      name=f"w1c_{j}_{i}")
                    base = i * W1S_ICOLS
                    eng = nc.scalar if (j == 0 and i in (1, 3)) else nc.sync
                    eng.dma_start(wt[:], w1S[:, base:base + W1S_ICOLS])
                    w_tiles.append(wt)

                for i in range(NI1):
                    wt = w_tiles[i]
                    ms = []
                    for P in range(7):
                        psum = ps.tile([128, 512], F32, tag=f"m{P}",
                                       name=f"g1_{j}_{i}_{P}")
                        for k in range(NK1):
                            nc.tensor.matmul(psum[:], wt[:, P, k], mov1(P, k),
                                             start=(k == 0), stop=(k == NK1 - 1))
                        ms.append(psum)
                    stA = sg.tile([128, 512], F32, tag="stA", name=f"stA{j}_{i}")
                    stB = sg.tile([128, 512], F32, tag="stB", name=f"stB{j}_{i}")
                    stC = sg.tile([128, 512], F32, tag="stC", name=f"stC{j}_{i}")
                    stD = sg.tile([128, 512], F32, tag="stD", name=f"stD{j}_{i}")
                    t0 = sgt.tile([128, 512], F32, tag="t0", name=f"t0_{j}_{i}")
                    t3 = sgt.tile([128, 512], F32, tag="t3", name=f"t3_{j}_{i}")
                    t4 = sgt.tile([128, 512], F32, tag="t4", name=f"t4_{j}_{i}")
                    # Any engine reads at most one PSUM input per op; copies
                    # of the twice-used products first, ordered so banks free
                    # in the next i's consumption order (m0 first). t0/t3 go
                    # through the lightly-loaded ScalarE to unload DVE.
                    nc.scalar.activation(t0[:], ms[0][:], IDENT)       # frees m0
                    nc.scalar.activation(t3[:], ms[3][:], IDENT)       # frees m3
                    nc.vector.tensor_copy(t4[:], ms[4][:])             # frees m4
                    nc.vector.tensor_add(stC[:], t3[:], ms[1][:])
                    nc.vector.tensor_sub(stD[:], t0[:], ms[1][:])      # frees m1
                    nc.vector.tensor_add(stB[:], t4[:], ms[2][:])
                    nc.vector.tensor_add(stD[:], stD[:], ms[2][:])     # frees m2
                    nc.vector.tensor_add(stD[:], stD[:], ms[5][:])     # frees m5
                    nc.vector.tensor_add(stA[:], t0[:], t3[:])
                    nc.vector.tensor_sub(stA[:], stA[:], t4[:])
                    nc.vector.tensor_add(stA[:], stA[:], ms[6][:])     # frees m6
                    nc.scalar.activation(hq[:, 16 + i, 0], stC[:], GELU,
                                         bias=bb_sb[:, 16 + i:17 + i], scale=1.0)
                    nc.scalar.activation(hq[:, i, 1], stB[:], GELU,
                                         bias=bb_sb[:, i:i + 1], scale=1.0)
                    nc.scalar.activation(hq[:, 16 + i, 1], stD[:], GELU,
                                         bias=bb_sb[:, 16 + i:17 + i], scale=1.0)
                    nc.scalar.activation(hq[:, i, 0], stA[:], GELU,
                                         bias=bb_sb[:, i:i + 1], scale=1.0)

            def gemm2(j):
                for d in range(ND2):
                    w2t = w2c_pool.tile([128, 32, 128], CDT, tag="w2c",
                                        name=f"w2c_{j}_{d}")
                    nc.scalar.dma_start(
                        w2t[:], w2L[:, d * NK2 * 128:(d + 1) * NK2 * 128])
                    for q in range(2):
                        # alternate with the (idle in this phase) m0 bank so
                        # the next group's start doesn't wait on the ACT read;
                        # the very last group drains in two column chunks
                        last = (j == 1 and d == ND2 - 1 and q == 1)
                        chunks = ((0, 256), (256, 512)) if last else ((0, 512),)
                        for ci, (c0, c1) in enumerate(chunks):
                            even = (d * 2 + q + ci) % 2 == 0
                            psum = ps.tile([128, c1 - c0], F32,
                                           tag=("g2" if even else "m0"),
                                           name=f"g2_{j}_{d}_{q}_{c0}")
                            for k in range(NK2):
                                nc.tensor.matmul(psum[:], w2t[:, k],
                                                 hq[:, k, q, c0:c1],
                                                 start=(k == 0),
                                                 stop=(k == NK2 - 1))
                            o_sb = op_pool.tile([128, c1 - c0], CDT, tag="o",
                                                name=f"o_{j}_{d}_{q}_{c0}")
                            nc.scalar.activation(o_sb[:], psum[:], IDENT,
                                                 bias=bb_sb[:, 32 + d:33 + d],
                                                 scale=1.0)
                            t0 = q * 1024 + j * 512
                            nc.scalar.dma_start(
                                yT[d * 128:(d + 1) * 128,
                                   t0 + c0:t0 + c1], o_sb[:])

            xr0, xc0 = load_x(0, first=True)
            gemm1(0, xr0, xc0)
            xr1, xc1 = load_x(1, first=False)   # hidden under gemm2(0)
            gemm2(0)
            gemm1(1, xr1, xc1)
            gemm2(1)

    nc.compile()
    return nc


def _get_compiled():
    global _compiled
    if _compiled is None:
        _compiled = _build()
    return _compiled


def _make_S(A):
    M, K = A.shape
    A11, A12 = A[:M // 2, :K // 2], A[:M // 2, K // 2:]
    A21, A22 = A[M // 2:, :K // 2], A[M // 2:, K // 2:]
    return np.stack([A11 + A22, A21 + A22, A11, A22,
                     A11 + A12, A21 - A11, A12 - A22]).astype(NP_CDT)


def _make_in_maps(x, w1, b1, w2, b2):
    in_maps = []
    for e in range(E):
        xe = x[e * T_PER_E:(e + 1) * T_PER_E].astype(np.float32)
        S1 = _make_S(w1[e].astype(np.float32))            # [7, 2048, 512]
        w1s = (S1.reshape(7, 16, 128, 4, 128)
               .transpose(4, 1, 0, 3, 2).reshape(128, -1))
        w2l = (w2[e].astype(NP_CDT).reshape(8, 128, 32, 128)
               .transpose(3, 0, 2, 1).reshape(128, -1))
        xq = xe.astype(NP_CDT).reshape(2, 2, 512, 8, 128)  # [q, j, c, k, p]
        pieces = np.stack([xq[0, :, :, 0:4, :], xq[1, :, :, 4:8, :],
                           xq[1, :, :, 0:4, :], xq[0, :, :, 4:8, :]])
        xl = pieces.transpose(4, 1, 0, 3, 2).reshape(128, -1)  # [p,j,pc,kk,c]
        bbe = np.concatenate([b1[e].reshape(32, 128).T,
                              b2[e].reshape(8, 128).T], axis=1).astype(np.float32)
        in_maps.append({
            "w1S": np.ascontiguousarray(w1s),
            "w2L": np.ascontiguousarray(w2l),
            "xL": np.ascontiguousarray(xl),
            "bb": np.ascontiguousarray(bbe),
            "wz": np.zeros((128, 256), NP_CDT),
        })
    return in_maps


def run(x, cnt, w1, b1, w2, b2, trace=False):
    nc = _get_compiled()
    in_maps = _make_in_maps(x, w1, b1, w2, b2)
    res = run_bass_kernel_spmd(nc, in_maps, core_ids=list(range(E)), trace=trace)
    outs = [res.results[e]["yT"].T for e in range(E)]
    y = np.concatenate(outs, axis=0).astype(np.float32)
    return y, res


def kernel(x, cnt, w1, b1, w2, b2):
    y, _ = run(x, cnt, w1, b1, w2, b2, trace=False)
    return y
